# revision 9
# baseline (speedup 1.0000x reference)
"""Trainium2 Bass kernel for the heatmap-BCE + Sinkhorn Wasserstein loss.

Strategy (8 NeuronCores, data-parallel over batch, 2 samples per core):
  - Heatmap render: separable gaussians via ACT (Square, Exp), outer-product
    accumulation on the PE (fp32 matmuls, K=68).
  - BCE: log/one-minus-log via ACT Ln (bit-identical tables to the jax
    reference lowering), clamp + fused row-reductions on the DVE; the final
    cross-partition reductions and tiny scalar combine happen on the host.
  - Sinkhorn: multiplicative form  u = 1/(K'v), v = 1/(K'^T u) with
    K' = exp(-C/eps)/N, which is algebraically the reference's log-domain
    iteration. Each half-step is one PE matvec + one DVE reciprocal.
  - Per-core partial sums are combined on the host (final all-reduce of the
    scalar loss).
"""

import numpy as np
from contextlib import ExitStack

import concourse.bacc as bacc
import concourse.mybir as mybir
import concourse.tile as tile
from concourse.bass_utils import run_bass_kernel_spmd

# problem constants (hardcoded per harness contract)
B, N, S = 16, 68, 256
N_CORES = 8
SPC = B // N_CORES            # samples per core = 2
SK_ITERS = 50
EPS_INV = 100.0               # 1/SK_EPS
LN_N = float(np.log(np.float32(N)))
BCE_COEF = 1000000.0
WAS_COEF = 2000.0

F32 = mybir.dt.float32
AF = mybir.ActivationFunctionType
ALU = mybir.AluOpType
AX = mybir.AxisListType

_STATE = {}


def _build():
    nc = bacc.Bacc("TRN2", target_bir_lowering=False, debug=False)

    coords = nc.dram_tensor("coords", [N, 8], F32, kind="ExternalInput")
    predT = nc.dram_tensor("predT", [34, N], F32, kind="ExternalInput")
    targT = nc.dram_tensor("targT", [34, N], F32, kind="ExternalInput")
    xs = nc.dram_tensor("xs", [N, S], F32, kind="ExternalInput")
    ident = nc.dram_tensor("ident", [N, N], F32, kind="ExternalInput")
    out_all = nc.dram_tensor("out_all", [128, 12], F32, kind="ExternalOutput")

    with tile.TileContext(nc) as tc, ExitStack() as ctx:
        sb = ctx.enter_context(tc.tile_pool(name="sb", bufs=1))
        gsb = ctx.enter_context(tc.tile_pool(name="gsb", bufs=3))
        bsb = ctx.enter_context(tc.tile_pool(name="bsb", bufs=2))
        lsb = ctx.enter_context(tc.tile_pool(name="lsb", bufs=2))
        php = ctx.enter_context(tc.tile_pool(name="php", bufs=2, space="PSUM"))
        parg = ctx.enter_context(tc.tile_pool(name="parg", bufs=1, space="PSUM"))
        plp = ctx.enter_context(tc.tile_pool(name="plp", bufs=2, space="PSUM"))
        pssp = ctx.enter_context(tc.tile_pool(name="pssp", bufs=1, space="PSUM"))

        # ---- phase 1: inputs needed by the sinkhorn chain ----
        c_t = sb.tile([N, 8], F32, tag="coords")
        nc.gpsimd.dma_start(c_t[:], coords.ap())
        pT_t = sb.tile([34, N], F32, tag="predT")
        nc.gpsimd.dma_start(pT_t[:], predT.ap())
        tT_t = sb.tile([34, N], F32, tag="targT")
        nc.gpsimd.dma_start(tT_t[:], targT.ap())
        idt = sb.tile([N, N], F32, tag="ident")
        nc.gpsimd.dma_start(idt[:], ident.ap())

        # ---- phase 2: K' = exp(-C/eps)/N and K'^T, M = K' * C ----
        pT2 = sb.tile([34, N], F32, tag="pT2")
        nc.scalar.activation(pT2[:], pT_t[:], AF.Copy, scale=-2.0)
        tT2 = sb.tile([34, N], F32, tag="tT2")
        nc.scalar.activation(tT2[:], tT_t[:], AF.Copy, scale=-2.0)
        sq8 = sb.tile([N, 8], F32, tag="sq8")
        nc.vector.tensor_tensor(sq8[:], c_t[:], c_t[:], ALU.mult)
        x2all = sb.tile([N, 4], F32, tag="x2all")  # |p0|^2,|p1|^2,|t0|^2,|t1|^2
        nc.vector.tensor_reduce(
            x2all[:], sq8[:].rearrange("p (a b) -> p a b", b=2), axis=AX.X, op=ALU.add
        )
        biasK = sb.tile([N, 2], F32, tag="biasK")  # -100*|x_i|^2 - ln N
        nc.vector.tensor_scalar(biasK[:], x2all[:, 0:2], -EPS_INV, -LN_N,
                                ALU.mult, ALU.add)
        biasT = sb.tile([N, 2], F32, tag="biasT")  # -100*|y_j|^2 - ln N
        nc.vector.tensor_scalar(biasT[:], x2all[:, 2:4], -EPS_INV, -LN_N,
                                ALU.mult, ALU.add)

        argK = parg.tile([N, 2 * N], F32, tag="argK")  # -2 x.y + |y_j|^2
        argT = parg.tile([N, 2 * N], F32, tag="argT")
        Kp = sb.tile([N, 2 * N], F32, tag="Kp")
        KTp = sb.tile([N, 2 * N], F32, tag="KTp")
        for s in range(SPC):
            sl = slice(N * s, N * (s + 1))
            rows = slice(32 * s, 32 * s + 2)   # sample rows at base partition 0/32
            nc.tensor.matmul(argK[:, sl], pT2[rows, :], tT_t[rows, :],
                             start=True, stop=False)
            nc.tensor.matmul(argK[:, sl],
                             x2all[:, 2 + s:3 + s].broadcast_to([N, N]), idt[:],
                             start=False, stop=True)
            nc.scalar.activation(Kp[:, sl], argK[:, sl], AF.Exp,
                                 bias=biasK[:, s:s + 1], scale=-EPS_INV)
            nc.tensor.matmul(argT[:, sl], tT2[rows, :], pT_t[rows, :],
                             start=True, stop=False)
            nc.tensor.matmul(argT[:, sl],
                             x2all[:, s:s + 1].broadcast_to([N, N]), idt[:],
                             start=False, stop=True)
            nc.scalar.activation(KTp[:, sl], argT[:, sl], AF.Exp,
                                 bias=biasT[:, s:s + 1], scale=-EPS_INV)

        Cfull = sb.tile([N, 2 * N], F32, tag="Cfull")
        Mker = sb.tile([N, 2 * N], F32, tag="Mker")
        for s in range(SPC):
            sl = slice(N * s, N * (s + 1))
            nc.vector.tensor_scalar(Cfull[:, sl], argK[:, sl],
                                    x2all[:, s:s + 1], None, ALU.add)
            nc.vector.tensor_tensor(Mker[:, sl], Kp[:, sl], Cfull[:, sl], ALU.mult)

        # ---- phase 3: multiplicative sinkhorn, 50 iterations ----
        accs = sb.tile([128, 12], F32, tag="accs")
        nc.vector.memset(accs[:], 0.0)

        v = lsb.tile([N, SPC], F32, tag="v")
        nc.vector.memset(v[:], 1.0)
        u = None
        for _ in range(SK_ITERS):
            pu = plp.tile([N, SPC], F32, tag="plp")
            for s in range(SPC):
                nc.tensor.matmul(pu[:, s:s + 1], KTp[:, N * s:N * (s + 1)],
                                 v[:, s:s + 1], start=True, stop=True)
            u = lsb.tile([N, SPC], F32, tag="u")
            nc.vector.reciprocal(u[:], pu[:])
            pv = plp.tile([N, SPC], F32, tag="plp")
            for s in range(SPC):
                nc.tensor.matmul(pv[:, s:s + 1], Kp[:, N * s:N * (s + 1)],
                                 u[:, s:s + 1], start=True, stop=True)
            v = lsb.tile([N, SPC], F32, tag="v")
            nc.vector.reciprocal(v[:], pv[:])

        # cost_s = u^T M v / N   (w = M^T u; then sum_j w_j * v_j on host)
        w = plp.tile([N, SPC], F32, tag="plp")
        for s in range(SPC):
            nc.tensor.matmul(w[:, s:s + 1], Mker[:, N * s:N * (s + 1)],
                             u[:, s:s + 1], start=True, stop=True)
        nc.vector.tensor_tensor(accs[0:N, 10:12], w[:], v[:], ALU.mult)

        # ---- phase 4: heatmaps ----
        xs_t = sb.tile([N, S], F32, tag="xs")
        nc.gpsimd.dma_start(xs_t[:], xs.ap())
        pxn = sb.tile([N, 8], F32, tag="pxn")  # -(coord * 255)
        nc.scalar.activation(pxn[:], c_t[:], AF.Copy, scale=-255.0)

        hm = []
        for m in range(4):  # pred_s0, pred_s1, targ_s0, targ_s1
            g = []
            for axis in range(2):
                col = 2 * m + axis
                sqg = gsb.tile([N, S], F32, tag="sqg")
                nc.scalar.activation(sqg[:], xs_t[:], AF.Square,
                                     bias=pxn[:, col:col + 1])
                gt = gsb.tile([N, S], F32, tag=f"g{axis}")
                nc.scalar.activation(gt[:], sqg[:], AF.Exp, scale=-0.03125)
                g.append(gt)
            gx, gy = g
            ph = php.tile([128, 2 * S], F32, tag="hm")
            nc.tensor.matmul(ph[:, 0:S], gy[:, 0:128], gx[:],
                             start=True, stop=True)
            nc.tensor.matmul(ph[:, S:2 * S], gy[:, 128:256], gx[:],
                             start=True, stop=True)
            hm_sb = sb.tile([128, 2 * S], F32, tag=f"hm{m}")
            nc.scalar.copy(hm_sb[:], ph[:])
            hm.append(hm_sb)

        # ---- phase 5: per-map pixel sums -> 1/(s+eps) scales ----
        rs4 = sb.tile([128, 4], F32, tag="rs4")
        for m in range(4):
            nc.vector.tensor_reduce(rs4[:, m:m + 1], hm[m][:], axis=AX.X, op=ALU.add)
        ones128 = sb.tile([128, 128], F32, tag="ones128")
        nc.vector.memset(ones128[:], 1.0)
        pss = pssp.tile([128, 4], F32, tag="pss")
        nc.tensor.matmul(pss[:], ones128[:], rs4[:], start=True, stop=True)
        nc.vector.tensor_copy(accs[:, 6:10], pss[:])
        ipe = sb.tile([128, 2], F32, tag="ipe")
        nc.vector.tensor_scalar(ipe[:], pss[:, 0:2], 1e-8, None, ALU.add)
        ipr = sb.tile([128, 2], F32, tag="ipr")
        nc.vector.reciprocal(ipr[:], ipe[:])
        negip = sb.tile([128, 2], F32, tag="negip")
        nc.vector.tensor_scalar(negip[:], ipr[:], -1.0, None, ALU.mult)

        # ---- phase 6: BCE pieces ----
        # W1 = sum(hm_t * max(ln(hm_p*ip), -100));  W2 = same with ln(1-hm_p*ip)
        # W3 = sum(max(ln(1-hm_p*ip), -100))
        for s in range(SPC):
            lp = bsb.tile([128, 2 * S], F32, tag="lp")
            nc.scalar.activation(lp[:], hm[s][:], AF.Ln, scale=ipr[:, s:s + 1])
            lpc = bsb.tile([128, 2 * S], F32, tag="lpc")
            nc.vector.tensor_scalar(lpc[:], lp[:], -100.0, None, ALU.max)
            lm = bsb.tile([128, 2 * S], F32, tag="lm")
            nc.scalar.activation(lm[:], hm[s][:], AF.Ln, scale=negip[:, s:s + 1],
                                 bias=1.0)
            lmc = bsb.tile([128, 2 * S], F32, tag="lmc")
            nc.vector.tensor_scalar(lmc[:], lm[:], -100.0, None, ALU.max, ALU.add,
                                    accum_out=accs[:, 3 * s + 2:3 * s + 3])
            scr1 = bsb.tile([128, 2 * S], F32, tag="scr1")
            nc.vector.scalar_tensor_tensor(
                scr1[:], hm[2 + s][:], 1.0, lpc[:], ALU.mult, ALU.mult,
                accum_out=accs[:, 3 * s:3 * s + 1])
            scr2 = bsb.tile([128, 2 * S], F32, tag="scr2")
            nc.vector.scalar_tensor_tensor(
                scr2[:], hm[2 + s][:], 1.0, lmc[:], ALU.mult, ALU.mult,
                accum_out=accs[:, 3 * s + 1:3 * s + 2])

        nc.gpsimd.dma_start(out_all.ap(), accs[:])

    nc.compile()
    return nc


def _get_nc():
    if "nc" not in _STATE:
        _STATE["nc"] = _build()
    return _STATE["nc"]


def _make_in_maps(pred_coord, target_coord):
    pred_coord = np.asarray(pred_coord, dtype=np.float32)
    target_coord = np.asarray(target_coord, dtype=np.float32)
    xs_np = np.broadcast_to(np.arange(S, dtype=np.float32), (N, S)).copy()
    id_np = np.eye(N, dtype=np.float32)
    in_maps = []
    for c in range(N_CORES):
        cols = []
        for s in range(SPC):
            cols.append(pred_coord[SPC * c + s])      # [N,2]
        for s in range(SPC):
            cols.append(target_coord[SPC * c + s])    # [N,2]
        coords_np = np.concatenate(cols, axis=1).astype(np.float32)  # [N,8]
        predT_np = np.zeros((34, N), dtype=np.float32)
        targT_np = np.zeros((34, N), dtype=np.float32)
        for s in range(SPC):
            predT_np[32 * s:32 * s + 2] = pred_coord[SPC * c + s].T
            targT_np[32 * s:32 * s + 2] = target_coord[SPC * c + s].T
        in_maps.append({
            "coords": coords_np,
            "predT": predT_np,
            "targT": targT_np,
            "xs": xs_np,
            "ident": id_np,
        })
    return in_maps


def _combine(results):
    bce_sum = 0.0
    cost_sum = 0.0
    for c in range(N_CORES):
        o = np.asarray(results[c]["out_all"], dtype=np.float64)
        srow = o[0, 6:10]
        for s in range(SPC):
            W1 = o[:, 3 * s].sum()
            W2 = o[:, 3 * s + 1].sum()
            W3 = o[:, 3 * s + 2].sum()
            s_t = srow[2 + s]
            bce_sum += (W1 - W2) / s_t + W3
            cost_sum += o[:, 10 + s].sum() / N
    bce = -bce_sum / (B * S * S)
    loss = BCE_COEF * bce + WAS_COEF * cost_sum / B
    return np.asarray(np.float32(loss))


def kernel(pred_coord, target_coord):
    nc = _get_nc()
    in_maps = _make_in_maps(pred_coord, target_coord)
    res = run_bass_kernel_spmd(nc, in_maps, list(range(N_CORES)))
    return _combine(res.results)


if __name__ == "__main__":
    rng = np.random.default_rng(0)
    p = rng.random((B, N, 2), dtype=np.float32)
    t = rng.random((B, N, 2), dtype=np.float32)
    print(kernel(p, t))
